# revision 1
# baseline (speedup 1.0000x reference)
"""Trainium2 Bass kernel for FNO projection (nn_FNOProjection_44616120271314).

Full inputs in, full output out. Data-parallel over batch: 8 batches per core
on 8 NeuronCores.

Device algorithm per core (8 batches, layout A = channels-on-partitions):
  h stored as 4 pair-tiles [128 = 2 batches x 64 ch, 4096 t].
  Per FNO layer:
    1. PE-transpose h chunks -> hT [t, rows]; truncated-DFT as one
       PSUM-accumulated matmul chain vs cos/sin matrix F (32 modes).
    2. Small PE-transposes rearrange HF -> [(comp,ch), (batch,mode)].
    3. Per-mode complex spectral matmul (weights as 128x128 real block form).
    4. Small PE-transposes rearrange OF -> [(comp,mode), (batch,ch)].
    5. iDFT matmul vs C + skip-conv (block-diag weights) accumulated into the
       same PSUM; fused bias+gelu on eviction (tanh gelu, matches jax default).
  Layer 3 evaluates only t = L-1; then the two projection matmuls.
All matmuls run in float32r (full-rate fp32 mode on the PE).
"""

import numpy as np

B, L, NIN, NST = 64, 4096, 8, 16
CIN, W, M, NL, PROJ = 24, 64, 32, 4, 128
NCORES = 8
BPC = B // NCORES          # batches per core
NP = BPC // 2              # pair-tiles per core

last_results = None        # test harness reads exec_time_ns from here


def _host_arrays(inputs):
    f32 = np.float32
    u, z = inputs["u"], inputs["z"]
    fc0_w, fc0_b = inputs["fc0_w"], inputs["fc0_b"]
    swr, swi = inputs["spec_wr"], inputs["spec_wi"]
    skw, skb = inputs["skip_w"], inputs["skip_b"]
    fc1_w, fc1_b = inputs["fc1_w"], inputs["fc1_b"]
    fc2_w, fc2_b = inputs["fc2_w"], inputs["fc2_b"]

    t_idx = np.arange(L, dtype=np.float64)[:, None]
    m_idx = np.arange(M, dtype=np.float64)[None, :]
    ang = 2.0 * np.pi * t_idx * m_idx / L
    F_full = np.concatenate([np.cos(ang), -np.sin(ang)], axis=1)   # [L, 64]
    F_host = np.ascontiguousarray(
        F_full.reshape(32, 128, 2 * M).transpose(1, 0, 2).reshape(128, 32 * 2 * M)
    ).astype(f32)

    Cr = (2.0 / L) * np.cos(ang.T)
    Cr[0, :] = 1.0 / L
    Ci = -(2.0 / L) * np.sin(ang.T)
    Ci[0, :] = 0.0
    C_host = np.concatenate([Cr, Ci], axis=0).astype(f32)          # [64, L]

    # spectral weights as per-mode real 128x128 blocks: rows (comp,i), cols (comp',o)
    W2 = np.zeros((NL, 128, 128, M), dtype=f32)
    W2[:, :64, :64, :] = swr
    W2[:, :64, 64:, :] = swi
    W2[:, 64:, :64, :] = -swi
    W2[:, 64:, 64:, :] = swr
    Wbig = np.ascontiguousarray(
        W2.transpose(0, 1, 3, 2).reshape(NL * 128, M * 128)
    ).astype(f32)                                                   # [512, 4096]

    Wbd = np.zeros((128, NL * 128), dtype=f32)                      # skip block-diag
    for l in range(NL):
        Wbd[:64, l * 128:l * 128 + 64] = skw[l]
        Wbd[64:, l * 128 + 64:l * 128 + 128] = skw[l]

    skipb = np.tile(skb.T, (2, 1)).astype(f32)                      # [128, NL]

    ID128 = np.eye(128, dtype=f32)

    zlift = (z @ fc0_w[NIN:] + fc0_b).astype(f32)                   # [B, W]

    shared = dict(
        F=F_host, C=C_host, Wbig=Wbig, Wbd=Wbd, skipb=skipb,
        id128=ID128,
        fc1w=fc1_w.astype(f32), fc1b=fc1_b.reshape(PROJ, 1).astype(f32),
        fc2w=fc2_w.astype(f32), fc2b=fc2_b.reshape(NST, 1).astype(f32),
    )

    # lift: K=33 block matmuls. Two pairs share a 33-row band (data rows of the
    # other pair hit a zero weight block); row 32 of each band is all-ones and
    # carries the per-pair bias (z-part of the lift + fc0_b) in the weights.
    bdU = np.zeros((16, 128), dtype=f32)
    bdU[:8, :64] = fc0_w[:NIN]
    bdU[8:, 64:] = fc0_w[:NIN]

    in_maps = []
    for core in range(NCORES):
        bs = core * BPC
        uT = np.zeros((66, L), dtype=f32)
        fc0u = np.zeros((128, 256), dtype=f32)
        for p in range(NP):
            band = 33 * (p // 2)
            half = 64 * (p // 2)
            cb = 128 * (p % 2)
            fc0u[half + 16 * (p % 2):half + 16 * (p % 2) + 16, cb:cb + 128] = bdU
            for b2 in range(2):
                b = bs + 2 * p + b2
                uT[band + 16 * (p % 2) + 8 * b2:
                   band + 16 * (p % 2) + 8 * b2 + 8] = u[b].T
                fc0u[half + 32, cb + 64 * b2:cb + 64 * b2 + 64] = zlift[b]
        uT[32] = 1.0
        uT[65] = 1.0
        m = {"uT": uT, "fc0u": fc0u}
        m.update(shared)
        in_maps.append(m)
    return in_maps


def _build(ctx, tc, io):
    import concourse.mybir as mybir

    nc = tc.nc
    f32 = mybir.dt.float32
    AF = mybir.ActivationFunctionType
    f32r = mybir.dt.float32r

    const = ctx.enter_context(tc.tile_pool(name="const", bufs=1))
    hpool = ctx.enter_context(tc.tile_pool(name="h", bufs=1))
    wpool = ctx.enter_context(tc.tile_pool(name="wbig", bufs=2))
    htp = ctx.enter_context(tc.tile_pool(name="ht", bufs=6))
    spool = ctx.enter_context(tc.tile_pool(name="small", bufs=2))
    pT = ctx.enter_context(tc.tile_pool(name="pT", bufs=3, space="PSUM"))
    pG = ctx.enter_context(tc.tile_pool(name="pG", bufs=2, space="PSUM"))
    phf = ctx.enter_context(tc.tile_pool(name="phf", bufs=1, space="PSUM"))

    def cload(name, shape, nchunks=1, dt=None):
        t = const.tile(shape, dt or f32, tag=name, name=name)
        cw = shape[1] // nchunks
        for c in range(nchunks):
            nc.sync.dma_start(t[:, c * cw:(c + 1) * cw],
                              io[name][:, c * cw:(c + 1) * cw])
        return t

    # lift-critical tensors first so the first matmuls start ASAP:
    # the small stationary (fc0u) must land before the streamed uT chunks
    fc0u_sb = cload("fc0u", [128, 256], dt=f32r)
    id128 = cload("id128", [128, 128], dt=f32r)
    uT_sb = const.tile([128, L], f32r, tag="uT", name="uT")
    for c in range(4):
        cs = 1024 * c
        nc.sync.dma_start(uT_sb[0:33, cs:cs + 1024], io["uT"][0:33, cs:cs + 1024])
        nc.sync.dma_start(uT_sb[64:97, cs:cs + 1024], io["uT"][33:66, cs:cs + 1024])
    F_sb = cload("F", [128, 32 * 2 * M], nchunks=2, dt=f32r)
    C_sb = cload("C", [2 * M, L], nchunks=2, dt=f32r)
    Wbd_sb = cload("Wbd", [128, NL * 128], dt=f32r)
    skipb_sb = cload("skipb", [128, NL])
    fc1w_sb = cload("fc1w", [W, PROJ], dt=f32r)
    fc1b_sb = cload("fc1b", [PROJ, 1])
    fc2w_sb = cload("fc2w", [PROJ, NST], dt=f32r)
    fc2b_sb = cload("fc2b", [NST, 1])

    def load_wbig(l):
        t = wpool.tile([128, M * 128], f32r, tag="wbig", name="wbig")
        for c in range(4):
            nc.sync.dma_start(
                t[:, 1024 * c:1024 * (c + 1)],
                io["Wbig"][128 * l:128 * (l + 1), 1024 * c:1024 * (c + 1)],
            )
        return t

    wbig_l = load_wbig(0)

    hA = [hpool.tile([128, L], f32r, tag=f"hA{p}", name=f"hA{p}")
          for p in range(NP)]

    # ---- lift: h0 = fc0ubd.T @ [u.T; ones] per pair (bias via ones-row) ----
    for g in range(4):                      # 1024-wide groups
        for p in range(NP):
            half = 64 * (p // 2)
            cb = 128 * (p % 2)
            ps = pG.tile([128, 1024], f32, tag="pg", name="ps")
            for hf_ in range(2):
                sl = 1024 * g + 512 * hf_
                nc.tensor.matmul(
                    ps[:, 512 * hf_:512 * (hf_ + 1)],
                    fc0u_sb[half:half + 33, cb:cb + 128],
                    uT_sb[half:half + 33, sl:sl + 512],
                    start=True, stop=True,
                )
            if (g + p) % 2 == 0:
                nc.vector.tensor_copy(hA[p][:, 1024 * g:1024 * (g + 1)], ps[:])
            else:
                nc.scalar.copy(hA[p][:, 1024 * g:1024 * (g + 1)], ps[:])

    # ---- FNO layers ----
    for l in range(NL):
        wbig_next = load_wbig(l + 1) if l < NL - 1 else None

        # 1) transpose h -> hT chunks; truncated DFT accumulated in PSUM
        psHF = phf.tile([2 * M, 512], f32, tag="hf", name="psHF")
        for tcd in range(32):
            psT = pT.tile([128, 512], f32r, tag="big", name="psT")
            for p in range(NP):
                nc.tensor.transpose(
                    psT[:, 128 * p:128 * (p + 1)],
                    hA[p][:, 128 * tcd:128 * (tcd + 1)], id128[:],
                )
            hTt = htp.tile([128, 512], f32r, tag="hT", name="hTt")
            if tcd % 2 == 0:
                nc.vector.tensor_copy(hTt[:], psT[:])
            else:
                nc.scalar.copy(hTt[:], psT[:])
            nc.tensor.matmul(
                psHF[:], F_sb[:, 64 * tcd:64 * (tcd + 1)], hTt[:],
                start=(tcd == 0), stop=(tcd == 31), skip_group_check=True,
            )

        HF_sb = spool.tile([2 * M, 512], f32r, tag="HF", name="HF_sb")
        nc.vector.tensor_copy(HF_sb[:, 0:256], psHF[:, 0:256])
        nc.scalar.copy(HF_sb[:, 256:512], psHF[:, 256:512])

        # 2) HF [(c,m),(b,i)] -> HFT [(c,i),(b,m)] via per-b transposes.
        # matmul PSUM outputs must start at partition 0, so the imag half goes
        # through SBUF staging + one partition-shifting DMA.
        HFT_sb = spool.tile([128, BPC * M], f32r, tag="HFT", name="HFT_sb")
        stH = spool.tile([64, BPC * M], f32r, tag="stH", name="stH")
        for b in range(BPC):
            psH = pT.tile([64, 64], f32r, tag="big", name="psH")
            nc.tensor.transpose(
                psH[:], HF_sb[:, 64 * b:64 * (b + 1)],
                id128[0:64, 0:64],
            )
            nc.vector.tensor_copy(HFT_sb[0:64, M * b:M * (b + 1)], psH[:, 0:M])
            nc.vector.tensor_copy(stH[:, M * b:M * (b + 1)], psH[:, M:2 * M])
            if b % 2 == 1:
                nc.sync.dma_start(HFT_sb[64:128, M * (b - 1):M * (b + 1)],
                                  stH[:, M * (b - 1):M * (b + 1)])

        hft3 = HFT_sb[:].rearrange("p (b m) -> p b m", m=M)

        # 3) per-mode complex spectral matmul -> OFT [(c',o),(b,m)]
        OFT_sb = spool.tile([128, BPC * M], f32r, tag="OFT", name="OFT_sb")
        oft3 = OFT_sb[:].rearrange("p (b m) -> p b m", m=M)
        for mg in range(M // 4):
            psS = pT.tile([128, 4 * BPC], f32, tag="big", name="psS")
            for j in range(4):
                m = 4 * mg + j
                nc.tensor.matmul(
                    psS[:, BPC * j:BPC * (j + 1)],
                    wbig_l[:, 128 * m:128 * (m + 1)], hft3[:, :, m],
                    start=True, stop=True,
                )
            nc.vector.tensor_copy(
                oft3[:, :, 4 * mg:4 * mg + 4],
                psS[:].rearrange("p (m b) -> p b m", b=BPC),
            )

        # 4) OFT -> OF2 [(c',m),(b,o)] via per-b transposes (imag via DMA)
        OF2_sb = spool.tile([2 * M, 512], f32r, tag="OF2", name="OF2_sb")
        stO = spool.tile([M, 512], f32r, tag="stO", name="stO")
        for b in range(BPC):
            psO = pT.tile([M, 128], f32r, tag="big", name="psO")
            nc.tensor.transpose(
                psO[:], OFT_sb[:, M * b:M * (b + 1)], id128[:],
            )
            nc.vector.tensor_copy(OF2_sb[0:M, 64 * b:64 * (b + 1)], psO[:, 0:64])
            nc.vector.tensor_copy(stO[:, 64 * b:64 * (b + 1)], psO[:, 64:128])
            if b % 4 == 3:
                nc.sync.dma_start(OF2_sb[M:2 * M, 64 * (b - 3):64 * (b + 1)],
                                  stO[:, 64 * (b - 3):64 * (b + 1)])

        if l < NL - 1:
            # 5) spec (iDFT) + skip accumulated in PSUM; bias+gelu on evict.
            # tcc-major so next layer's transposes unblock chunk by chunk.
            for g in range(4):
                for p in range(NP):
                    ps = pG.tile([128, 1024], f32, tag="pg", name="ps")
                    for hf_ in range(2):
                        sl = 1024 * g + 512 * hf_
                        nc.tensor.matmul(
                            ps[:, 512 * hf_:512 * (hf_ + 1)],
                            OF2_sb[:, 128 * p:128 * (p + 1)],
                            C_sb[:, sl:sl + 512],
                            start=True, stop=False,
                        )
                        nc.tensor.matmul(
                            ps[:, 512 * hf_:512 * (hf_ + 1)],
                            Wbd_sb[:, 128 * l:128 * (l + 1)],
                            hA[p][:, sl:sl + 512],
                            start=False, stop=True,
                        )
                    nc.scalar.activation(
                        hA[p][:, 1024 * g:1024 * (g + 1)], ps[:],
                        AF.Gelu_apprx_tanh, bias=skipb_sb[:, l:l + 1],
                    )
        else:
            # layer 3: only t = L-1 is needed downstream
            psL = pT.tile([128, 8 * NP], f32, tag="big", name="psL")
            for p in range(NP):
                nc.tensor.matmul(
                    psL[:, 8 * p:8 * (p + 1)], OF2_sb[:, 128 * p:128 * (p + 1)],
                    C_sb[:, L - 8:L], start=True, stop=False,
                )
                nc.tensor.matmul(
                    psL[:, 8 * p:8 * (p + 1)], Wbd_sb[:, 128 * l:128 * (l + 1)],
                    hA[p][:, L - 8:L], start=False, stop=True,
                )
            hl2 = spool.tile([128, NP], f32r, tag="hl2", name="hl2")
            nc.scalar.activation(hl2[:], psL[:, 7::8], AF.Identity,
                                 bias=skipb_sb[:, l:l + 1])
            hlast = spool.tile([W, BPC], f32r, tag="hlast", name="hlast")
            hl3 = hlast[:].rearrange("i (p b) -> i p b", b=2)
            nc.sync.dma_start(hl3[:, :, 0], hl2[0:64, :])
            nc.sync.dma_start(hl3[:, :, 1], hl2[64:128, :])

        wbig_l = wbig_next

    # ---- projection at t = L-1 ----
    psQ = pT.tile([PROJ, BPC], f32, tag="big", name="psQ")
    nc.tensor.matmul(psQ[:], fc1w_sb[:], hlast[:], start=True, stop=True)
    q_sb = spool.tile([PROJ, BPC], f32r, tag="q", name="q_sb")
    nc.scalar.activation(q_sb[:], psQ[:], AF.Gelu_apprx_tanh, bias=fc1b_sb[:])

    psO2 = pT.tile([NST, BPC], f32, tag="big", name="psO2")
    nc.tensor.matmul(psO2[:], fc2w_sb[:], q_sb[:], start=True, stop=True)
    out_sb = spool.tile([NST, BPC], f32, tag="out", name="out_sb")
    nc.scalar.activation(out_sb[:], psO2[:], AF.Identity, bias=fc2b_sb[:])
    nc.sync.dma_start(io["out_t"][:], out_sb[:])


def _make_runner(nc, in_maps):
    """Build a reusable jitted SPMD callable with device-resident inputs.

    Mirrors bass2jax.run_bass_via_pjrt but keeps the jit + device arrays so
    repeated calls can be timed without retransfer/retrace.
    """
    import jax
    import numpy as _np
    from jax.sharding import Mesh, PartitionSpec
    from jax.experimental.shard_map import shard_map

    import concourse.mybir as mybir
    from concourse import bass2jax

    bass2jax.install_neuronx_cc_hook()
    n_cores = len(in_maps)
    partition_name = (nc.partition_id_tensor.name
                      if nc.partition_id_tensor else None)
    in_names, out_names, out_avals, zero_outs = [], [], [], []
    for alloc in nc.m.functions[0].allocations:
        if not isinstance(alloc, mybir.MemoryLocationSet):
            continue
        name = alloc.memorylocations[0].name
        if alloc.kind == "ExternalInput":
            if name != partition_name:
                in_names.append(name)
        elif alloc.kind == "ExternalOutput":
            shape = tuple(alloc.tensor_shape)
            dtype = mybir.dt.np(alloc.dtype)
            out_names.append(name)
            out_avals.append(jax.core.ShapedArray(shape, dtype))
            zero_outs.append(_np.zeros(shape, dtype))
    n_params = len(in_names)
    n_outs = len(out_avals)
    all_in_names = in_names + out_names + ([partition_name] if partition_name else [])
    donate = tuple(range(n_params, n_params + n_outs))

    def _body(*args):
        operands = list(args)
        if partition_name is not None:
            operands.append(bass2jax.partition_id_tensor())
        outs = bass2jax._bass_exec_p.bind(
            *operands,
            out_avals=tuple(out_avals),
            in_names=tuple(all_in_names),
            out_names=tuple(out_names),
            lowering_input_output_aliases=(),
            sim_require_finite=True,
            sim_require_nnan=True,
            nc=nc,
        )
        return tuple(outs)

    devices = jax.devices()[:n_cores]
    mesh = Mesh(np.asarray(devices), ("core",))
    sharded = jax.jit(
        shard_map(_body, mesh=mesh,
                  in_specs=(PartitionSpec("core"),) * (n_params + n_outs),
                  out_specs=(PartitionSpec("core"),) * n_outs,
                  check_rep=False),
        donate_argnums=donate, keep_unused=True,
    )
    concat_in = [
        np.concatenate([np.asarray(in_maps[c][nm]) for c in range(n_cores)], axis=0)
        for nm in in_names
    ]
    dev_in = [jax.device_put(a) for a in concat_in]
    jax.block_until_ready(dev_in)

    def run():
        zeros = [np.zeros((n_cores * z.shape[0], *z.shape[1:]), z.dtype)
                 for z in zero_outs]
        out = sharded(*dev_in, *zeros)
        jax.block_until_ready(out)
        return out

    def unpack(out_arrs):
        return [
            {nm: np.asarray(out_arrs[i]).reshape(n_cores, *out_avals[i].shape)[c]
             for i, nm in enumerate(out_names)}
            for c in range(n_cores)
        ]

    return run, unpack


def _build_nc(in_maps, body):
    from contextlib import ExitStack

    import concourse.bacc as bacc
    import concourse.mybir as mybir
    import concourse.tile as tile

    nc = bacc.Bacc("TRN2", target_bir_lowering=False, debug=False)
    io = {}
    f32r_names = {"uT", "fc0u", "F", "C", "Wbd", "Wbig", "id128", "fc1w", "fc2w"}
    for name, arr in in_maps[0].items():
        dt_ = (mybir.dt.float32r if name in f32r_names else mybir.dt.float32)
        io[name] = nc.dram_tensor(name, list(arr.shape), dt_,
                                  kind="ExternalInput").ap()
    io["out_t"] = nc.dram_tensor("out_t", [NST, BPC], mybir.dt.float32,
                                 kind="ExternalOutput").ap()
    with tile.TileContext(nc) as tc, ExitStack() as ctx:
        body(ctx, tc, io)
    nc.compile()
    return nc


def benchmark(inputs, iters=30):
    """Time repeated on-device executions; subtract a no-op kernel baseline."""
    import time

    in_maps = _host_arrays(inputs)
    nc = _build_nc(in_maps, _build)
    run, unpack = _make_runner(nc, in_maps)
    run()  # warm
    run()
    t0 = time.perf_counter()
    for _ in range(iters):
        out = run()
    t_full = (time.perf_counter() - t0) / iters

    # dispatch-overhead baseline: trivial kernel, same I/O signature
    def _tiny(ctx, tc, io):
        import concourse.mybir as mybir
        nc2 = tc.nc
        pool = ctx.enter_context(tc.tile_pool(name="t", bufs=1))
        t = pool.tile([NST, BPC], mybir.dt.float32, tag="o", name="o")
        nc2.sync.dma_start(t[:, 0:1], io["fc2b"][:])
        nc2.sync.dma_start(io["out_t"][:, 0:1], t[:, 0:1])

    nc0 = _build_nc(in_maps, _tiny)
    run0, _ = _make_runner(nc0, in_maps)
    run0(); run0()
    t0 = time.perf_counter()
    for _ in range(iters):
        run0()
    t_base = (time.perf_counter() - t0) / iters
    return t_full, t_base, unpack(out)


def kernel(**inputs):
    global last_results

    in_maps = _host_arrays(inputs)
    nc = _build_nc(in_maps, _build)

    import os
    from concourse.bass_utils import run_bass_kernel_spmd
    res = run_bass_kernel_spmd(
        nc, in_maps, list(range(NCORES)),
        trace=bool(os.environ.get("BASS_TRACE")),
    )
    last_results = res
    out = np.concatenate([r["out_t"].T for r in res.results], axis=0)
    return np.ascontiguousarray(out.astype(np.float32))



# revision 2
# speedup vs baseline: 7.2511x; 7.2511x over previous
"""Trainium2 Bass kernel for FNO projection (nn_FNOProjection_44616120271314).

Full inputs in, full output out. Data-parallel over batch: 8 batches per core
on 8 NeuronCores. fp16 data path (fp32 PSUM accumulation), tuned from NTFF
hardware traces:
  - h state in one fp16 tile [128, 4*4096]; per-layer truncated DFT via PE
    transposes + one PSUM-accumulated matmul chain (fp16 runs 1 cyc/row vs
    fp32r's 2, and small-N matmuls get compiler FWL).
  - spectral mode matmuls split into K=64 real/imag pairs (no partition-shift
    staging for HFT); iDFT C operand zero-padded to K=128 (64-partition
    operands stream at half rate).
  - iDFT+skip weight-blocked in 4-chunk groups over 4 single-bank PSUM tiles
    (keeps the PE HAM-warm; 2-deep 2-bank rotation ran cold the whole phase).
  - SBUF-to-SBUF shift/staging DMAs on SWDGE (HWDGE queue adds ~10us latency).
"""

import numpy as np

B, L, NIN, NST = 64, 4096, 8, 16
CIN, W, M, NL, PROJ = 24, 64, 32, 4, 128
NCORES = 8
BPC = B // NCORES          # batches per core
NP = BPC // 2              # pair-tiles per core

last_results = None        # test harness reads exec_time_ns from here

F16 = np.float16


def _host_arrays(inputs):
    f32 = np.float32
    u, z = inputs["u"], inputs["z"]
    fc0_w, fc0_b = inputs["fc0_w"], inputs["fc0_b"]
    swr, swi = inputs["spec_wr"], inputs["spec_wi"]
    skw, skb = inputs["skip_w"], inputs["skip_b"]
    fc1_w, fc1_b = inputs["fc1_w"], inputs["fc1_b"]
    fc2_w, fc2_b = inputs["fc2_w"], inputs["fc2_b"]

    t_idx = np.arange(L, dtype=np.float64)[:, None]
    m_idx = np.arange(M, dtype=np.float64)[None, :]
    ang = 2.0 * np.pi * t_idx * m_idx / L
    F_full = np.concatenate([np.cos(ang), -np.sin(ang)], axis=1)   # [L, 64]
    F_host = np.ascontiguousarray(
        F_full.reshape(32, 128, 2 * M).transpose(1, 0, 2).reshape(128, 32 * 2 * M)
    ).astype(F16)

    Cr = (2.0 / L) * np.cos(ang.T)
    Cr[0, :] = 1.0 / L
    Ci = -(2.0 / L) * np.sin(ang.T)
    Ci[0, :] = 0.0
    C_host = np.zeros((128, L), dtype=F16)                         # K=128 padded
    C_host[0:64] = np.concatenate([Cr, Ci], axis=0).astype(F16)

    # spectral weights as per-mode real 128x128 blocks: rows (comp,i), cols (comp',o)
    W2 = np.zeros((NL, 128, 128, M), dtype=f32)
    W2[:, :64, :64, :] = swr
    W2[:, :64, 64:, :] = swi
    W2[:, 64:, :64, :] = -swi
    W2[:, 64:, 64:, :] = swr
    Wbig = np.ascontiguousarray(
        W2.transpose(0, 1, 3, 2).reshape(NL * 128, M * 128)
    ).astype(F16)                                                   # [512, 4096]

    Wbd = np.zeros((128, NL * 128), dtype=f32)                      # skip block-diag
    for l in range(NL):
        Wbd[:64, l * 128:l * 128 + 64] = skw[l]
        Wbd[64:, l * 128 + 64:l * 128 + 128] = skw[l]
    Wbd = Wbd.astype(F16)

    skipb = np.tile(skb.T, (2, 1)).astype(f32)                      # [128, NL]

    ID128 = np.eye(128, dtype=F16)

    zlift = (z @ fc0_w[NIN:] + fc0_b).astype(f32)                   # [B, W]

    shared = dict(
        F=F_host, C=C_host, Wbig=Wbig, Wbd=Wbd, skipb=skipb,
        id128=ID128,
        fc1w=fc1_w.astype(f32), fc1b=fc1_b.reshape(PROJ, 1).astype(f32),
        fc2w=fc2_w.astype(f32), fc2b=fc2_b.reshape(NST, 1).astype(f32),
    )

    # lift: K=33 block matmuls. Two pairs share a 33-row band (data rows of the
    # other pair hit a zero weight block); row 32 of each band is all-ones and
    # carries the per-pair bias (z-part of the lift + fc0_b) in the weights.
    bdU = np.zeros((16, 128), dtype=f32)
    bdU[:8, :64] = fc0_w[:NIN]
    bdU[8:, 64:] = fc0_w[:NIN]

    in_maps = []
    for core in range(NCORES):
        bs = core * BPC
        uT = np.zeros((66, L), dtype=F16)
        fc0u = np.zeros((128, 256), dtype=F16)
        for p in range(NP):
            band = 33 * (p // 2)
            half = 64 * (p // 2)
            cb = 128 * (p % 2)
            fc0u[half + 16 * (p % 2):half + 16 * (p % 2) + 16, cb:cb + 128] = bdU
            for b2 in range(2):
                b = bs + 2 * p + b2
                uT[band + 16 * (p % 2) + 8 * b2:
                   band + 16 * (p % 2) + 8 * b2 + 8] = u[b].T
                fc0u[half + 32, cb + 64 * b2:cb + 64 * b2 + 64] = zlift[b]
        uT[32] = 1.0
        uT[65] = 1.0
        m = {"uT": uT, "fc0u": fc0u}
        m.update(shared)
        in_maps.append(m)
    return in_maps


def _build(ctx, tc, io):
    import concourse.mybir as mybir

    nc = tc.nc
    f32 = mybir.dt.float32
    f16 = mybir.dt.float16
    AF = mybir.ActivationFunctionType
    f32r = mybir.dt.float32r

    const = ctx.enter_context(tc.tile_pool(name="const", bufs=1))
    hpool = ctx.enter_context(tc.tile_pool(name="h", bufs=1))
    wpool = ctx.enter_context(tc.tile_pool(name="wbig", bufs=2))
    spool = ctx.enter_context(tc.tile_pool(name="small", bufs=2))
    htp = ctx.enter_context(tc.tile_pool(name="ht", bufs=6))
    pT = ctx.enter_context(tc.tile_pool(name="pT", bufs=3, space="PSUM"))
    pG = ctx.enter_context(tc.tile_pool(name="pG", bufs=4, space="PSUM"))
    phf = ctx.enter_context(tc.tile_pool(name="phf", bufs=1, space="PSUM"))

    def cload(name, shape, nchunks=1, dt=None):
        t = const.tile(shape, dt or f32, tag=name, name=name)
        cw = shape[1] // nchunks
        for c in range(nchunks):
            nc.sync.dma_start(t[:, c * cw:(c + 1) * cw],
                              io[name][:, c * cw:(c + 1) * cw])
        return t

    # lift-critical tensors first so the first matmuls start ASAP
    fc0u_sb = cload("fc0u", [128, 256], dt=f16)
    id128 = cload("id128", [128, 128], dt=f16)
    uT_sb = const.tile([128, L], f16, tag="uT", name="uT")
    for c in range(4):
        cs = 1024 * c
        nc.sync.dma_start(uT_sb[0:33, cs:cs + 1024], io["uT"][0:33, cs:cs + 1024])
        nc.scalar.dma_start(uT_sb[64:97, cs:cs + 1024], io["uT"][33:66, cs:cs + 1024])
    F_sb = cload("F", [128, 32 * 2 * M], dt=f16)
    C_sb = cload("C", [128, L], nchunks=2, dt=f16)
    Wbd_sb = cload("Wbd", [128, NL * 128], dt=f16)
    skipb_sb = cload("skipb", [128, NL])
    fc1w_sb = cload("fc1w", [W, PROJ], dt=f32r)
    fc1b_sb = cload("fc1b", [PROJ, 1])
    fc2w_sb = cload("fc2w", [PROJ, NST], dt=f32r)
    fc2b_sb = cload("fc2b", [NST, 1])

    def load_wbig(l):
        tr = wpool.tile([64, M * 128], f16, tag="wbigr", name="wbigr")
        ti = wpool.tile([64, M * 128], f16, tag="wbigi", name="wbigi")
        for c in range(2):
            nc.sync.dma_start(
                tr[:, 2048 * c:2048 * (c + 1)],
                io["Wbig"][128 * l:128 * l + 64, 2048 * c:2048 * (c + 1)],
            )
            nc.sync.dma_start(
                ti[:, 2048 * c:2048 * (c + 1)],
                io["Wbig"][128 * l + 64:128 * (l + 1), 2048 * c:2048 * (c + 1)],
            )
        return tr, ti

    wbig_l = load_wbig(0)

    # h state: one bf16 tile, pair-major; transposed copy hT, pair-major with
    # 128-wide t-chunks inside (xbar layout: hT[tt, p*L + 128c + n] = h[n, 128c+tt])
    hA = hpool.tile([128, NP * L], f16, tag="hA", name="hA")

    # ---- lift: h0 = fc0ubd.T @ [u.T; ones] per pair (bias via ones-row) ----
    for p in range(NP):
        for g in range(8):                  # 512-wide groups
            half = 64 * (p // 2)
            cb = 128 * (p % 2)
            ps = pG.tile([128, 512], f32, tag="pg", name="ps")
            sl = 512 * g
            nc.tensor.matmul(
                ps[:],
                fc0u_sb[half:half + 33, cb:cb + 128],
                uT_sb[half:half + 33, sl:sl + 512],
                start=True, stop=True,
            )
            if (g + p) % 2 == 0:
                nc.vector.tensor_copy(hA[:, p * L + sl:p * L + sl + 512], ps[:])
            else:
                nc.scalar.copy(hA[:, p * L + sl:p * L + sl + 512], ps[:])

    # ---- FNO layers ----
    for l in range(NL):
        wbig_next = load_wbig(l + 1) if l < NL - 1 else None

        # 1) transpose h -> hT via xbar DMA (8 half-pair transposes), then
        #    truncated DFT accumulated in PSUM
        psHF = phf.tile([2 * M, 512], f32, tag="hf", name="psHF")
        for tcd in range(32):
            psT = pT.tile([128, 512], f16, tag="big", name="psT")
            for p in range(NP):
                nc.tensor.transpose(
                    psT[:, 128 * p:128 * (p + 1)],
                    hA[:, p * L + 128 * tcd:p * L + 128 * (tcd + 1)], id128[:],
                )
            hTt = htp.tile([128, 512], f16, tag="hT", name="hTt")
            nc.vector.tensor_copy(hTt[:], psT[:])
            nc.tensor.matmul(
                psHF[:], F_sb[:, 64 * tcd:64 * (tcd + 1)], hTt[:],
                start=(tcd == 0), stop=(tcd == 31), skip_group_check=True,
            )

        HF_sb = spool.tile([2 * M, 512], f16, tag="HF", name="HF_sb")
        nc.vector.tensor_copy(HF_sb[:, 0:256], psHF[:, 0:256])
        nc.scalar.copy(HF_sb[:, 256:512], psHF[:, 256:512])

        # 2) HF [(c,m),(b,i)] -> HFT [(c,i),(b,m)] via per-b transposes.
        HFTr = spool.tile([64, BPC * M], f16, tag="HFTr", name="HFTr")
        HFTi = spool.tile([64, BPC * M], f16, tag="HFTi", name="HFTi")
        for b in range(BPC):
            psH = pT.tile([64, 64], f16, tag="big", name="psH")
            nc.tensor.transpose(
                psH[:], HF_sb[:, 64 * b:64 * (b + 1)],
                id128[0:64, 0:64],
            )
            nc.vector.tensor_copy(HFTr[:, M * b:M * (b + 1)], psH[:, 0:M])
            nc.vector.tensor_copy(HFTi[:, M * b:M * (b + 1)], psH[:, M:2 * M])

        hftr3 = HFTr[:].rearrange("p (b m) -> p b m", m=M)
        hfti3 = HFTi[:].rearrange("p (b m) -> p b m", m=M)

        # 3) per-mode complex spectral matmul -> OFT [(c',o),(b,m)]
        OFT_sb = spool.tile([128, BPC * M], f16, tag="OFT", name="OFT_sb")
        oft3 = OFT_sb[:].rearrange("p (b m) -> p b m", m=M)
        for mg in range(M // 4):
            psS = pT.tile([128, 4 * BPC], f32, tag="big", name="psS")
            for j in range(4):
                m = 4 * mg + j
                nc.tensor.matmul(
                    psS[:, BPC * j:BPC * (j + 1)],
                    wbig_l[0][:, 128 * m:128 * (m + 1)], hftr3[:, :, m],
                    start=True, stop=False, skip_group_check=True,
                )
                nc.tensor.matmul(
                    psS[:, BPC * j:BPC * (j + 1)],
                    wbig_l[1][:, 128 * m:128 * (m + 1)], hfti3[:, :, m],
                    start=False, stop=True, skip_group_check=True,
                )
            nc.vector.tensor_copy(
                oft3[:, :, 4 * mg:4 * mg + 4],
                psS[:].rearrange("p (m b) -> p b m", b=BPC),
            )

        # 4) OFT -> OF2 [(c',m),(b,o)] via per-b transposes (imag via DMA)
        OF2_sb = spool.tile([128, 512], f16, tag="OF2", name="OF2_sb")
        nc.vector.memset(OF2_sb[64:128, :], 0.0)
        stO = spool.tile([M, 512], f16, tag="stO", name="stO")
        for b in range(BPC):
            psO = pT.tile([M, 128], f16, tag="big", name="psO")
            nc.tensor.transpose(
                psO[:], OFT_sb[:, M * b:M * (b + 1)], id128[:],
            )
            nc.vector.tensor_copy(OF2_sb[0:M, 64 * b:64 * (b + 1)], psO[:, 0:64])
            nc.vector.tensor_copy(stO[:, 64 * b:64 * (b + 1)], psO[:, 64:128])
            if b % 2 == 1:
                nc.gpsimd.dma_start(OF2_sb[M:2 * M, 64 * (b - 1):64 * (b + 1)],
                                    stO[:, 64 * (b - 1):64 * (b + 1)])

        if l < NL - 1:
            # 5) spec (iDFT) + skip accumulated in PSUM; bias+gelu on evict.
            for p in range(NP):
                for gb in range(2):          # 4-chunk blocks: C x4 then skip x4
                    tiles = []
                    for gk in range(4):
                        sl = 512 * (4 * gb + gk)
                        ps = pG.tile([128, 512], f32, tag="pg", name="ps")
                        tiles.append((ps, sl))
                        nc.tensor.matmul(
                            ps[:],
                            OF2_sb[:, 128 * p:128 * (p + 1)],
                            C_sb[:, sl:sl + 512],
                            start=True, stop=False, skip_group_check=True,
                        )
                    for ps, sl in tiles:
                        nc.tensor.matmul(
                            ps[:],
                            Wbd_sb[:, 128 * l:128 * (l + 1)],
                            hA[:, p * L + sl:p * L + sl + 512],
                            start=False, stop=True, skip_group_check=True,
                        )
                        nc.scalar.activation(
                            hA[:, p * L + sl:p * L + sl + 512], ps[:],
                            AF.Gelu_apprx_tanh, bias=skipb_sb[:, l:l + 1],
                        )
        else:
            # layer 3: only t = L-1 is needed downstream
            psL = pT.tile([128, 8 * NP], f32, tag="big", name="psL")
            for p in range(NP):
                nc.tensor.matmul(
                    psL[:, 8 * p:8 * (p + 1)], OF2_sb[:, 128 * p:128 * (p + 1)],
                    C_sb[:, L - 8:L], start=True, stop=False,
                    skip_group_check=True,
                )
                nc.tensor.matmul(
                    psL[:, 8 * p:8 * (p + 1)], Wbd_sb[:, 128 * l:128 * (l + 1)],
                    hA[:, p * L + L - 8:p * L + L],
                    start=False, stop=True, skip_group_check=True,
                )
            hl2 = spool.tile([128, NP], f32r, tag="hl2", name="hl2")
            nc.scalar.activation(hl2[:], psL[:, 7::8], AF.Identity,
                                 bias=skipb_sb[:, l:l + 1])
            hlast = spool.tile([W, BPC], f32r, tag="hlast", name="hlast")
            hl3 = hlast[:].rearrange("i (p b) -> i p b", b=2)
            nc.gpsimd.dma_start(hl3[:, :, 0], hl2[0:64, :])
            nc.gpsimd.dma_start(hl3[:, :, 1], hl2[64:128, :])

        wbig_l = wbig_next

    # ---- projection at t = L-1 ----
    psQ = pT.tile([PROJ, BPC], f32, tag="big", name="psQ")
    nc.tensor.matmul(psQ[:], fc1w_sb[:], hlast[:], start=True, stop=True)
    q_sb = spool.tile([PROJ, BPC], f32r, tag="q", name="q_sb")
    nc.scalar.activation(q_sb[:], psQ[:], AF.Gelu_apprx_tanh, bias=fc1b_sb[:])

    psO2 = pT.tile([NST, BPC], f32, tag="big", name="psO2")
    nc.tensor.matmul(psO2[:], fc2w_sb[:], q_sb[:], start=True, stop=True)
    out_sb = spool.tile([NST, BPC], f32, tag="out", name="out_sb")
    nc.scalar.activation(out_sb[:], psO2[:], AF.Identity, bias=fc2b_sb[:])
    nc.gpsimd.dma_start(io["out_t"][:], out_sb[:])


def _make_runner(nc, in_maps):
    """Build a reusable jitted SPMD callable with device-resident inputs.

    Mirrors bass2jax.run_bass_via_pjrt but keeps the jit + device arrays so
    repeated calls can be timed without retransfer/retrace.
    """
    import jax
    import numpy as _np
    from jax.sharding import Mesh, PartitionSpec
    from jax.experimental.shard_map import shard_map

    import concourse.mybir as mybir
    from concourse import bass2jax

    bass2jax.install_neuronx_cc_hook()
    n_cores = len(in_maps)
    partition_name = (nc.partition_id_tensor.name
                      if nc.partition_id_tensor else None)
    in_names, out_names, out_avals, zero_outs = [], [], [], []
    for alloc in nc.m.functions[0].allocations:
        if not isinstance(alloc, mybir.MemoryLocationSet):
            continue
        name = alloc.memorylocations[0].name
        if alloc.kind == "ExternalInput":
            if name != partition_name:
                in_names.append(name)
        elif alloc.kind == "ExternalOutput":
            shape = tuple(alloc.tensor_shape)
            dtype = mybir.dt.np(alloc.dtype)
            out_names.append(name)
            out_avals.append(jax.core.ShapedArray(shape, dtype))
            zero_outs.append(_np.zeros(shape, dtype))
    n_params = len(in_names)
    n_outs = len(out_avals)
    all_in_names = in_names + out_names + ([partition_name] if partition_name else [])
    donate = tuple(range(n_params, n_params + n_outs))

    def _body(*args):
        operands = list(args)
        if partition_name is not None:
            operands.append(bass2jax.partition_id_tensor())
        outs = bass2jax._bass_exec_p.bind(
            *operands,
            out_avals=tuple(out_avals),
            in_names=tuple(all_in_names),
            out_names=tuple(out_names),
            lowering_input_output_aliases=(),
            sim_require_finite=True,
            sim_require_nnan=True,
            nc=nc,
        )
        return tuple(outs)

    devices = jax.devices()[:n_cores]
    mesh = Mesh(np.asarray(devices), ("core",))
    sharded = jax.jit(
        shard_map(_body, mesh=mesh,
                  in_specs=(PartitionSpec("core"),) * (n_params + n_outs),
                  out_specs=(PartitionSpec("core"),) * n_outs,
                  check_rep=False),
        donate_argnums=donate, keep_unused=True,
    )
    concat_in = [
        np.concatenate([np.asarray(in_maps[c][nm]) for c in range(n_cores)], axis=0)
        for nm in in_names
    ]
    dev_in = [jax.device_put(a) for a in concat_in]
    jax.block_until_ready(dev_in)

    def run():
        zeros = [np.zeros((n_cores * z.shape[0], *z.shape[1:]), z.dtype)
                 for z in zero_outs]
        out = sharded(*dev_in, *zeros)
        jax.block_until_ready(out)
        return out

    def unpack(out_arrs):
        return [
            {nm: np.asarray(out_arrs[i]).reshape(n_cores, *out_avals[i].shape)[c]
             for i, nm in enumerate(out_names)}
            for c in range(n_cores)
        ]

    return run, unpack


def _build_nc(in_maps, body):
    from contextlib import ExitStack

    import concourse.bacc as bacc
    import concourse.mybir as mybir
    import concourse.tile as tile

    nc = bacc.Bacc("TRN2", target_bir_lowering=False, debug=False)
    io = {}
    f32r_names = {"fc1w", "fc2w"}
    bf16_names = {"uT", "fc0u", "C", "Wbig", "id128", "F", "Wbd"}
    for name, arr in in_maps[0].items():
        if name in bf16_names:
            dt_ = mybir.dt.float16
        elif name in f32r_names:
            dt_ = mybir.dt.float32r
        else:
            dt_ = mybir.dt.float32
        io[name] = nc.dram_tensor(name, list(arr.shape), dt_,
                                  kind="ExternalInput").ap()
    io["out_t"] = nc.dram_tensor("out_t", [NST, BPC], mybir.dt.float32,
                                 kind="ExternalOutput").ap()
    with tile.TileContext(nc) as tc, ExitStack() as ctx:
        body(ctx, tc, io)
    nc.compile()
    return nc


def benchmark(inputs, iters=30):
    """Time repeated on-device executions; subtract a no-op kernel baseline."""
    import time

    in_maps = _host_arrays(inputs)
    nc = _build_nc(in_maps, _build)
    run, unpack = _make_runner(nc, in_maps)
    run()  # warm
    run()
    t0 = time.perf_counter()
    for _ in range(iters):
        out = run()
    t_full = (time.perf_counter() - t0) / iters

    # dispatch-overhead baseline: trivial kernel, same I/O signature
    def _tiny(ctx, tc, io):
        import concourse.mybir as mybir
        nc2 = tc.nc
        pool = ctx.enter_context(tc.tile_pool(name="t", bufs=1))
        t = pool.tile([NST, BPC], mybir.dt.float32, tag="o", name="o")
        nc2.sync.dma_start(t[:, 0:1], io["fc2b"][:])
        nc2.sync.dma_start(io["out_t"][:, 0:1], t[:, 0:1])

    nc0 = _build_nc(in_maps, _tiny)
    run0, _ = _make_runner(nc0, in_maps)
    run0(); run0()
    t0 = time.perf_counter()
    for _ in range(iters):
        run0()
    t_base = (time.perf_counter() - t0) / iters
    return t_full, t_base, unpack(out)


def kernel(**inputs):
    global last_results

    in_maps = _host_arrays(inputs)
    nc = _build_nc(in_maps, _build)

    import os
    from concourse.bass_utils import run_bass_kernel_spmd
    res = run_bass_kernel_spmd(
        nc, in_maps, list(range(NCORES)),
        trace=bool(os.environ.get("BASS_TRACE")),
    )
    last_results = res
    out = np.concatenate([r["out_t"].T for r in res.results], axis=0)
    return np.ascontiguousarray(out.astype(np.float32))


# revision 3
# speedup vs baseline: 8.7118x; 1.2014x over previous
"""Trainium2 Bass kernel for FNO projection (nn_FNOProjection_44616120271314).

Full inputs in, full output out. Data-parallel over batch: 8 batches per core
on 8 NeuronCores. fp16 data path (fp32 PSUM accumulation), tuned from NTFF
hardware traces:
  - h state in one fp16 tile [128, 4*4096]; per-layer truncated DFT via PE
    transposes + one PSUM-accumulated matmul chain (fp16 runs 1 cyc/row vs
    fp32r's 2, and small-N matmuls get compiler FWL).
  - spectral mode matmuls split into K=64 real/imag pairs (no partition-shift
    staging for HFT); iDFT C operand zero-padded to K=128 (64-partition
    operands stream at half rate).
  - iDFT+skip weight-blocked in 4-chunk groups over 4 single-bank PSUM tiles
    (keeps the PE HAM-warm; 2-deep 2-bank rotation ran cold the whole phase).
  - SBUF-to-SBUF shift/staging DMAs on SWDGE (HWDGE queue adds ~10us latency).
"""

import numpy as np

B, L, NIN, NST = 64, 4096, 8, 16
CIN, W, M, NL, PROJ = 24, 64, 32, 4, 128
NCORES = 8
BPC = B // NCORES          # batches per core
NP = BPC // 2              # pair-tiles per core

last_results = None        # test harness reads exec_time_ns from here

F16 = np.float16


def _host_arrays(inputs):
    f32 = np.float32
    u, z = inputs["u"], inputs["z"]
    fc0_w, fc0_b = inputs["fc0_w"], inputs["fc0_b"]
    swr, swi = inputs["spec_wr"], inputs["spec_wi"]
    skw, skb = inputs["skip_w"], inputs["skip_b"]
    fc1_w, fc1_b = inputs["fc1_w"], inputs["fc1_b"]
    fc2_w, fc2_b = inputs["fc2_w"], inputs["fc2_b"]

    t_idx = np.arange(L, dtype=np.float64)[:, None]
    m_idx = np.arange(M, dtype=np.float64)[None, :]
    ang = 2.0 * np.pi * t_idx * m_idx / L
    F_full = np.concatenate([np.cos(ang), -np.sin(ang)], axis=1)   # [L, 64]
    F_host = np.ascontiguousarray(
        F_full.reshape(32, 128, 2 * M).transpose(1, 0, 2).reshape(128, 32 * 2 * M)
    ).astype(F16)

    Cr = (2.0 / L) * np.cos(ang.T)
    Cr[0, :] = 1.0 / L
    Ci = -(2.0 / L) * np.sin(ang.T)
    Ci[0, :] = 0.0
    C_host = np.zeros((128, L), dtype=F16)                         # K=128 padded
    C_host[0:64] = np.concatenate([Cr, Ci], axis=0).astype(F16)

    # spectral weights as per-mode real 128x128 blocks: rows (comp,i), cols (comp',o)
    W2 = np.zeros((NL, 128, 128, M), dtype=f32)
    W2[:, :64, :64, :] = swr
    W2[:, :64, 64:, :] = swi
    W2[:, 64:, :64, :] = -swi
    W2[:, 64:, 64:, :] = swr
    Wbig = np.ascontiguousarray(
        W2.transpose(0, 1, 3, 2).reshape(NL * 128, M * 128)
    ).astype(F16)                                                   # [512, 4096]

    Wbd = np.zeros((128, NL * 128), dtype=f32)                      # skip block-diag
    for l in range(NL):
        Wbd[:64, l * 128:l * 128 + 64] = skw[l]
        Wbd[64:, l * 128 + 64:l * 128 + 128] = skw[l]
    Wbd = Wbd.astype(F16)

    skipb = np.tile(skb.T, (2, 1)).astype(f32)                      # [128, NL]

    ID128 = np.eye(128, dtype=F16)

    zlift = (z @ fc0_w[NIN:] + fc0_b).astype(f32)                   # [B, W]

    shared = dict(
        F=F_host, C=C_host, Wbig=Wbig, Wbd=Wbd, skipb=skipb,
        id128=ID128,
        fc1w=fc1_w.astype(f32), fc1b=fc1_b.reshape(PROJ, 1).astype(f32),
        fc2w=fc2_w.astype(f32), fc2b=fc2_b.reshape(NST, 1).astype(f32),
    )

    # lift: K=33 block matmuls. Two pairs share a 33-row band (data rows of the
    # other pair hit a zero weight block); row 32 of each band is all-ones and
    # carries the per-pair bias (z-part of the lift + fc0_b) in the weights.
    bdU = np.zeros((16, 128), dtype=f32)
    bdU[:8, :64] = fc0_w[:NIN]
    bdU[8:, 64:] = fc0_w[:NIN]

    in_maps = []
    for core in range(NCORES):
        bs = core * BPC
        uT = np.zeros((66, L), dtype=F16)
        fc0u = np.zeros((128, 256), dtype=F16)
        for p in range(NP):
            band = 33 * (p // 2)
            half = 64 * (p // 2)
            cb = 128 * (p % 2)
            fc0u[half + 16 * (p % 2):half + 16 * (p % 2) + 16, cb:cb + 128] = bdU
            for b2 in range(2):
                b = bs + 2 * p + b2
                uT[band + 16 * (p % 2) + 8 * b2:
                   band + 16 * (p % 2) + 8 * b2 + 8] = u[b].T
                fc0u[half + 32, cb + 64 * b2:cb + 64 * b2 + 64] = zlift[b]
        uT[32] = 1.0
        uT[65] = 1.0
        m = {"uT": uT, "fc0u": fc0u}
        m.update(shared)
        in_maps.append(m)
    return in_maps


def _build(ctx, tc, io):
    import concourse.mybir as mybir

    nc = tc.nc
    f32 = mybir.dt.float32
    f16 = mybir.dt.float16
    AF = mybir.ActivationFunctionType
    f32r = mybir.dt.float32r

    const = ctx.enter_context(tc.tile_pool(name="const", bufs=1))
    hpool = ctx.enter_context(tc.tile_pool(name="h", bufs=1))
    wpool = ctx.enter_context(tc.tile_pool(name="wbig", bufs=2))
    spool = ctx.enter_context(tc.tile_pool(name="small", bufs=2))
    htp = ctx.enter_context(tc.tile_pool(name="ht", bufs=6))
    pT = ctx.enter_context(tc.tile_pool(name="pT", bufs=3, space="PSUM"))
    pG = ctx.enter_context(tc.tile_pool(name="pG", bufs=4, space="PSUM"))
    phf = ctx.enter_context(tc.tile_pool(name="phf", bufs=1, space="PSUM"))

    def cload(name, shape, nchunks=1, dt=None):
        t = const.tile(shape, dt or f32, tag=name, name=name)
        cw = shape[1] // nchunks
        for c in range(nchunks):
            nc.sync.dma_start(t[:, c * cw:(c + 1) * cw],
                              io[name][:, c * cw:(c + 1) * cw])
        return t

    # lift-critical tensors first so the first matmuls start ASAP
    fc0u_sb = cload("fc0u", [128, 256], dt=f16)
    id128 = cload("id128", [128, 128], dt=f16)
    uT_sb = const.tile([128, L], f16, tag="uT", name="uT")
    for c in range(4):
        cs = 1024 * c
        nc.sync.dma_start(uT_sb[0:33, cs:cs + 1024], io["uT"][0:33, cs:cs + 1024])
        nc.scalar.dma_start(uT_sb[64:97, cs:cs + 1024], io["uT"][33:66, cs:cs + 1024])
    F_sb = cload("F", [128, 32 * 2 * M], dt=f16)
    C_sb = cload("C", [128, L], nchunks=2, dt=f16)
    Wbd_sb = cload("Wbd", [128, NL * 128], dt=f16)
    skipb_sb = cload("skipb", [128, NL])
    fc1w_sb = cload("fc1w", [W, PROJ], dt=f32r)
    fc1b_sb = cload("fc1b", [PROJ, 1])
    fc2w_sb = cload("fc2w", [PROJ, NST], dt=f32r)
    fc2b_sb = cload("fc2b", [NST, 1])

    def load_wbig(l):
        tr = wpool.tile([64, M * 128], f16, tag="wbigr", name="wbigr")
        ti = wpool.tile([64, M * 128], f16, tag="wbigi", name="wbigi")
        for c in range(2):
            nc.sync.dma_start(
                tr[:, 2048 * c:2048 * (c + 1)],
                io["Wbig"][128 * l:128 * l + 64, 2048 * c:2048 * (c + 1)],
            )
            nc.sync.dma_start(
                ti[:, 2048 * c:2048 * (c + 1)],
                io["Wbig"][128 * l + 64:128 * (l + 1), 2048 * c:2048 * (c + 1)],
            )
        return tr, ti

    wbig_l = load_wbig(0)

    # h state: one bf16 tile, pair-major; transposed copy hT, pair-major with
    # 128-wide t-chunks inside (xbar layout: hT[tt, p*L + 128c + n] = h[n, 128c+tt])
    hA = hpool.tile([128, NP * L], f16, tag="hA", name="hA")

    # ---- lift: h0 = fc0ubd.T @ [u.T; ones] per pair (bias via ones-row) ----
    for p in range(NP):
        for g in range(8):                  # 512-wide groups
            half = 64 * (p // 2)
            cb = 128 * (p % 2)
            ps = pG.tile([128, 512], f32, tag="pg", name="ps")
            sl = 512 * g
            nc.tensor.matmul(
                ps[:],
                fc0u_sb[half:half + 33, cb:cb + 128],
                uT_sb[half:half + 33, sl:sl + 512],
                start=True, stop=True,
            )
            if (g + p) % 2 == 0:
                nc.vector.tensor_copy(hA[:, p * L + sl:p * L + sl + 512], ps[:])
            else:
                nc.scalar.copy(hA[:, p * L + sl:p * L + sl + 512], ps[:])

    # ---- FNO layers ----
    for l in range(NL):
        wbig_next = load_wbig(l + 1) if l < NL - 1 else None

        # 1) transpose h -> hT via xbar DMA (8 half-pair transposes), then
        #    truncated DFT accumulated in PSUM
        psHF = phf.tile([2 * M, 512], f32, tag="hf", name="psHF")
        for tcd in range(32):
            psT = pT.tile([128, 512], f16, tag="big", name="psT")
            for p in range(NP):
                nc.tensor.transpose(
                    psT[:, 128 * p:128 * (p + 1)],
                    hA[:, p * L + 128 * tcd:p * L + 128 * (tcd + 1)], id128[:],
                )
            hTt = htp.tile([128, 512], f16, tag="hT", name="hTt")
            nc.vector.tensor_copy(hTt[:], psT[:])
            nc.tensor.matmul(
                psHF[:], F_sb[:, 64 * tcd:64 * (tcd + 1)], hTt[:],
                start=(tcd == 0), stop=(tcd == 31), skip_group_check=True,
            )

        HF_sb = spool.tile([2 * M, 512], f16, tag="HF", name="HF_sb")
        nc.vector.tensor_copy(HF_sb[:, 0:256], psHF[:, 0:256])
        nc.scalar.copy(HF_sb[:, 256:512], psHF[:, 256:512])

        # 2) HF [(c,m),(b,i)] -> HFT [(c,i),(b,m)] via per-b transposes.
        HFTr = spool.tile([64, BPC * M], f16, tag="HFTr", name="HFTr")
        HFTi = spool.tile([64, BPC * M], f16, tag="HFTi", name="HFTi")
        for b in range(BPC):
            psH = pT.tile([64, 64], f16, tag="big", name="psH")
            nc.tensor.transpose(
                psH[:], HF_sb[:, 64 * b:64 * (b + 1)],
                id128[0:64, 0:64],
            )
            nc.vector.tensor_copy(HFTr[:, M * b:M * (b + 1)], psH[:, 0:M])
            nc.vector.tensor_copy(HFTi[:, M * b:M * (b + 1)], psH[:, M:2 * M])

        hftr3 = HFTr[:].rearrange("p (b m) -> p b m", m=M)
        hfti3 = HFTi[:].rearrange("p (b m) -> p b m", m=M)

        # 3) per-mode complex spectral matmul -> OFT [(c',o),(b,m)]
        OFT_sb = spool.tile([128, BPC * M], f16, tag="OFT", name="OFT_sb")
        oft3 = OFT_sb[:].rearrange("p (b m) -> p b m", m=M)
        for mg in range(M // 4):
            psS = pT.tile([128, 4 * BPC], f32, tag="big", name="psS")
            for j in range(4):
                m = 4 * mg + j
                nc.tensor.matmul(
                    psS[:, BPC * j:BPC * (j + 1)],
                    wbig_l[0][:, 128 * m:128 * (m + 1)], hftr3[:, :, m],
                    start=True, stop=False, skip_group_check=True,
                )
                nc.tensor.matmul(
                    psS[:, BPC * j:BPC * (j + 1)],
                    wbig_l[1][:, 128 * m:128 * (m + 1)], hfti3[:, :, m],
                    start=False, stop=True, skip_group_check=True,
                )
            nc.vector.tensor_copy(
                oft3[:, :, 4 * mg:4 * mg + 4],
                psS[:].rearrange("p (m b) -> p b m", b=BPC),
            )

        # 4) OFT -> OF2 [(c',m),(b,o)] via per-b transposes (imag via DMA)
        OF2_sb = spool.tile([128, 512], f16, tag="OF2", name="OF2_sb")
        nc.vector.memset(OF2_sb[64:128, :], 0.0)
        stO = spool.tile([M, 512], f16, tag="stO", name="stO")
        for b in range(BPC):
            psO = pT.tile([M, 128], f16, tag="big", name="psO")
            nc.tensor.transpose(
                psO[:], OFT_sb[:, M * b:M * (b + 1)], id128[:],
            )
            nc.vector.tensor_copy(OF2_sb[0:M, 64 * b:64 * (b + 1)], psO[:, 0:64])
            nc.vector.tensor_copy(stO[:, 64 * b:64 * (b + 1)], psO[:, 64:128])
            if b % 2 == 1:
                nc.gpsimd.dma_start(OF2_sb[M:2 * M, 64 * (b - 1):64 * (b + 1)],
                                    stO[:, 64 * (b - 1):64 * (b + 1)])

        if l < NL - 1:
            # 5) spec (iDFT) + skip accumulated in PSUM; bias+gelu on evict.
            for p in range(NP):
                for gb in range(2):          # 4-chunk blocks: skip x4 then C x4
                    tiles = []
                    for gk in range(4):
                        sl = 512 * (4 * gb + gk)
                        ps = pG.tile([128, 512], f32, tag="pg", name="ps")
                        tiles.append((ps, sl))
                        nc.tensor.matmul(
                            ps[:],
                            Wbd_sb[:, 128 * l:128 * (l + 1)],
                            hA[:, p * L + sl:p * L + sl + 512],
                            start=True, stop=False, skip_group_check=True,
                        )
                    for ps, sl in tiles:
                        nc.tensor.matmul(
                            ps[:],
                            OF2_sb[:, 128 * p:128 * (p + 1)],
                            C_sb[:, sl:sl + 512],
                            start=False, stop=True, skip_group_check=True,
                        )
                        nc.scalar.activation(
                            hA[:, p * L + sl:p * L + sl + 512], ps[:],
                            AF.Gelu_apprx_tanh, bias=skipb_sb[:, l:l + 1],
                        )
        else:
            # layer 3: only t = L-1 is needed downstream
            psL = pT.tile([128, 8 * NP], f32, tag="big", name="psL")
            for p in range(NP):
                nc.tensor.matmul(
                    psL[:, 8 * p:8 * (p + 1)], OF2_sb[:, 128 * p:128 * (p + 1)],
                    C_sb[:, L - 8:L], start=True, stop=False,
                    skip_group_check=True,
                )
                nc.tensor.matmul(
                    psL[:, 8 * p:8 * (p + 1)], Wbd_sb[:, 128 * l:128 * (l + 1)],
                    hA[:, p * L + L - 8:p * L + L],
                    start=False, stop=True, skip_group_check=True,
                )
            hl2 = spool.tile([128, NP], f32r, tag="hl2", name="hl2")
            nc.scalar.activation(hl2[:], psL[:, 7::8], AF.Identity,
                                 bias=skipb_sb[:, l:l + 1])
            hlast = spool.tile([W, BPC], f32r, tag="hlast", name="hlast")
            hl3 = hlast[:].rearrange("i (p b) -> i p b", b=2)
            nc.gpsimd.dma_start(hl3[:, :, 0], hl2[0:64, :])
            nc.gpsimd.dma_start(hl3[:, :, 1], hl2[64:128, :])

        wbig_l = wbig_next

    # ---- projection at t = L-1 ----
    psQ = pT.tile([PROJ, BPC], f32, tag="big", name="psQ")
    nc.tensor.matmul(psQ[:], fc1w_sb[:], hlast[:], start=True, stop=True)
    q_sb = spool.tile([PROJ, BPC], f32r, tag="q", name="q_sb")
    nc.scalar.activation(q_sb[:], psQ[:], AF.Gelu_apprx_tanh, bias=fc1b_sb[:])

    psO2 = pT.tile([NST, BPC], f32, tag="big", name="psO2")
    nc.tensor.matmul(psO2[:], fc2w_sb[:], q_sb[:], start=True, stop=True)
    out_sb = spool.tile([NST, BPC], f32, tag="out", name="out_sb")
    nc.scalar.activation(out_sb[:], psO2[:], AF.Identity, bias=fc2b_sb[:])
    nc.gpsimd.dma_start(io["out_t"][:], out_sb[:])


def _make_runner(nc, in_maps):
    """Build a reusable jitted SPMD callable with device-resident inputs.

    Mirrors bass2jax.run_bass_via_pjrt but keeps the jit + device arrays so
    repeated calls can be timed without retransfer/retrace.
    """
    import jax
    import numpy as _np
    from jax.sharding import Mesh, PartitionSpec
    from jax.experimental.shard_map import shard_map

    import concourse.mybir as mybir
    from concourse import bass2jax

    bass2jax.install_neuronx_cc_hook()
    n_cores = len(in_maps)
    partition_name = (nc.partition_id_tensor.name
                      if nc.partition_id_tensor else None)
    in_names, out_names, out_avals, zero_outs = [], [], [], []
    for alloc in nc.m.functions[0].allocations:
        if not isinstance(alloc, mybir.MemoryLocationSet):
            continue
        name = alloc.memorylocations[0].name
        if alloc.kind == "ExternalInput":
            if name != partition_name:
                in_names.append(name)
        elif alloc.kind == "ExternalOutput":
            shape = tuple(alloc.tensor_shape)
            dtype = mybir.dt.np(alloc.dtype)
            out_names.append(name)
            out_avals.append(jax.core.ShapedArray(shape, dtype))
            zero_outs.append(_np.zeros(shape, dtype))
    n_params = len(in_names)
    n_outs = len(out_avals)
    all_in_names = in_names + out_names + ([partition_name] if partition_name else [])
    donate = tuple(range(n_params, n_params + n_outs))

    def _body(*args):
        operands = list(args)
        if partition_name is not None:
            operands.append(bass2jax.partition_id_tensor())
        outs = bass2jax._bass_exec_p.bind(
            *operands,
            out_avals=tuple(out_avals),
            in_names=tuple(all_in_names),
            out_names=tuple(out_names),
            lowering_input_output_aliases=(),
            sim_require_finite=True,
            sim_require_nnan=True,
            nc=nc,
        )
        return tuple(outs)

    devices = jax.devices()[:n_cores]
    mesh = Mesh(np.asarray(devices), ("core",))
    sharded = jax.jit(
        shard_map(_body, mesh=mesh,
                  in_specs=(PartitionSpec("core"),) * (n_params + n_outs),
                  out_specs=(PartitionSpec("core"),) * n_outs,
                  check_rep=False),
        donate_argnums=donate, keep_unused=True,
    )
    concat_in = [
        np.concatenate([np.asarray(in_maps[c][nm]) for c in range(n_cores)], axis=0)
        for nm in in_names
    ]
    dev_in = [jax.device_put(a) for a in concat_in]
    jax.block_until_ready(dev_in)

    def run():
        zeros = [np.zeros((n_cores * z.shape[0], *z.shape[1:]), z.dtype)
                 for z in zero_outs]
        out = sharded(*dev_in, *zeros)
        jax.block_until_ready(out)
        return out

    def unpack(out_arrs):
        return [
            {nm: np.asarray(out_arrs[i]).reshape(n_cores, *out_avals[i].shape)[c]
             for i, nm in enumerate(out_names)}
            for c in range(n_cores)
        ]

    return run, unpack


def _build_nc(in_maps, body):
    from contextlib import ExitStack

    import concourse.bacc as bacc
    import concourse.mybir as mybir
    import concourse.tile as tile

    nc = bacc.Bacc("TRN2", target_bir_lowering=False, debug=False)
    io = {}
    f32r_names = {"fc1w", "fc2w"}
    bf16_names = {"uT", "fc0u", "C", "Wbig", "id128", "F", "Wbd"}
    for name, arr in in_maps[0].items():
        if name in bf16_names:
            dt_ = mybir.dt.float16
        elif name in f32r_names:
            dt_ = mybir.dt.float32r
        else:
            dt_ = mybir.dt.float32
        io[name] = nc.dram_tensor(name, list(arr.shape), dt_,
                                  kind="ExternalInput").ap()
    io["out_t"] = nc.dram_tensor("out_t", [NST, BPC], mybir.dt.float32,
                                 kind="ExternalOutput").ap()
    with tile.TileContext(nc) as tc, ExitStack() as ctx:
        body(ctx, tc, io)
    nc.compile()
    return nc


def benchmark(inputs, iters=30):
    """Time repeated on-device executions; subtract a no-op kernel baseline."""
    import time

    in_maps = _host_arrays(inputs)
    nc = _build_nc(in_maps, _build)
    run, unpack = _make_runner(nc, in_maps)
    run()  # warm
    run()
    t0 = time.perf_counter()
    for _ in range(iters):
        out = run()
    t_full = (time.perf_counter() - t0) / iters

    # dispatch-overhead baseline: trivial kernel, same I/O signature
    def _tiny(ctx, tc, io):
        import concourse.mybir as mybir
        nc2 = tc.nc
        pool = ctx.enter_context(tc.tile_pool(name="t", bufs=1))
        t = pool.tile([NST, BPC], mybir.dt.float32, tag="o", name="o")
        nc2.sync.dma_start(t[:, 0:1], io["fc2b"][:])
        nc2.sync.dma_start(io["out_t"][:, 0:1], t[:, 0:1])

    nc0 = _build_nc(in_maps, _tiny)
    run0, _ = _make_runner(nc0, in_maps)
    run0(); run0()
    t0 = time.perf_counter()
    for _ in range(iters):
        run0()
    t_base = (time.perf_counter() - t0) / iters
    return t_full, t_base, unpack(out)


def kernel(**inputs):
    global last_results

    in_maps = _host_arrays(inputs)
    nc = _build_nc(in_maps, _build)

    import os
    from concourse.bass_utils import run_bass_kernel_spmd
    res = run_bass_kernel_spmd(
        nc, in_maps, list(range(NCORES)),
        trace=bool(os.environ.get("BASS_TRACE")),
    )
    last_results = res
    out = np.concatenate([r["out_t"].T for r in res.results], axis=0)
    return np.ascontiguousarray(out.astype(np.float32))
